# revision 37
# baseline (speedup 1.0000x reference)
"""Trainium2 Bass kernel for nn_Cross_attention_89730456748795.

Full-input contract: kernel(**inputs) takes the unsharded inputs of
reference.setup_inputs() and returns the full [4, 128, 64, 64] output.

Sharding: the model has 4 batches x 2 cross-attention branches = 8
independent attention instances; one per NeuronCore (pure data parallel,
no collectives). Each core:
  - runs both depthwise convs (3x3 reflect-pad-1 and 5x5 reflect-pad-2)
    as diagonal-weight PE matmuls over a reflect-padded image, with
    LeakyReLU+bias+per-channel sums fused into one ScalarE op per tile,
  - folds both GroupNorms into the QKV projection weights (scale into the
    stationary operand, bias via a tiny N=1 matmul),
  - computes one full 4096x4096 attention in scores-transposed layout
    (softmax denominators come free from a ones-column appended to V^T),
  - output-projects, divides by the softmax sums, adds bias + residual.
"""

import math

import numpy as np

B, C, H, W = 4, 64, 64, 64
HW = H * W  # 4096
PAD = 2
HP = H + 2 * PAD  # 68
N_CORES = 8
GROUPS = 16
EPS = 1e-5
NBLK = 8  # q blocks of 512
BLK = 512
NCHUNK = 32  # k chunks of 128
CHUNK = 128
# chunk-pairs whose exp runs on DVE via the Schraudolph bit trick
# (i16 = round(A16*arg + B16) bitcast to bf16 ~= exp(arg), 1.8% rms),
# offloading ~1/4 of the exp stream from the bottleneck ScalarE
DVE_CPS = (2, 5, 8, 11, 14)
A16 = 128.0 / math.log(2.0)
B16 = 16248.5

# conv tap groups: (dy, dx0, paired). Pairs cover (dy,dx0),(dy,dx0+1).
TAP_GROUPS = [(dy, dx0, True) for dy in range(5) for dx0 in (0, 2)] + [
    (dy, 4, False) for dy in range(5)
]
NG = len(TAP_GROUPS)  # 15

_compiled = None
last_res = None  # BassKernelResults of the most recent run (for profiling)


def _build(dbg=False, loop_n=None):
    import concourse.tile as tile
    from concourse import bacc, mybir

    f32 = mybir.dt.float32
    f32r = mybir.dt.float32r
    Alu = mybir.AluOpType
    Act = mybir.ActivationFunctionType

    nc = bacc.Bacc("TRN2", target_bir_lowering=False, debug=False,
                   num_devices=N_CORES)

    def din(name, shape, dt=f32):
        return nc.dram_tensor(name, shape, dt, kind="ExternalInput").ap()

    x_in = din("x_in", [C, HP * HP], f32r)
    conv_wt = din("conv_wt", [128, NG * 128], f32r)
    conv_b = din("conv_b", [128, 1])
    gn_w = din("gn_w", [128, 1])
    gn_b = din("gn_b", [128, 1])
    gmat = din("gmat", [128, 128])
    wkk = din("wkk", [128, 128])
    wqq = din("wqq", [128, 128])
    wvr = din("wvr", [128, 128])
    proj_b = din("proj_b", [C, 1])
    ident = din("ident", [128, 128])
    out_d = nc.dram_tensor("out", [C, HW], f32, kind="ExternalOutput").ap()
    dbg_d = {}
    if dbg:
        for nm, shp in [("d_xc", [128, HW]), ("d_kk", [128, HW]),
                        ("d_qq", [128, HW]), ("d_vres", [128, HW]),
                        ("d_vaug", [128, NCHUNK * 65]),
                        ("d_scale", [128, 1]), ("d_beff", [128, 1]),
                        ("d_rs", [1, BLK]),
                        ("d_rbs", [C, BLK]), ("d_et", [128, 2 * BLK])]:
            dbg_d[nm] = nc.dram_tensor(nm, shp, f32,
                                       kind="ExternalOutput").ap()

    with tile.TileContext(nc) as tc:
        # ---- persistent SBUF tensors ----
        persist = tc.alloc_tile_pool(name="persist", bufs=1)

        def T(shape, name, dt=f32):
            return persist.tile(shape, dt, tag=name, name=name)

        x2 = T([128, HP, HP], "x2", f32r)  # top: padded, bottom: +1 shift
        cw = T([128, NG * 128], "cw", f32r)
        xc = T([128, HW], "xc", f32r)  # conv+leaky out (br1|br2)
        bf16 = mybir.dt.bfloat16
        kk = T([128, HW], "kk", bf16)  # kf duplicated on both halves
        qq = T([128, HW], "qq", bf16)  # qf duplicated on both halves
        vres = T([128, HW], "vres")  # rows 0-63 res, 64-127 vf
        vaug = T([128, NCHUNK * 65], "vaug", bf16)
        gmat_s = T([128, 128], "gmat_s")
        wkk_s = T([128, 128], "wkk_s")
        wqq_s = T([128, 128], "wqq_s")
        wvr_s = T([128, 128], "wvr_s")
        wkk_e = T([128, 128], "wkk_e", f32r)
        wqq_e = T([128, 128], "wqq_e", f32r)
        wvr_e = T([128, 128], "wvr_e", f32r)
        proj_b_s = T([C, 1], "proj_b_s")
        ident_s = T([128, 128], "ident_s")
        conv_b_s = T([128, 1], "conv_b_s")
        gn_w_s = T([128, 1], "gn_w_s")
        gn_b_s = T([128, 1], "gn_b_s")
        s1p = T([128, NBLK], "s1p")
        s2p = T([128, NBLK], "s2p")
        stats = T([128, 2], "stats")
        mean_s = T([128, 1], "mean_s")
        negmean = T([128, 1], "negmean")
        var_s = T([128, 1], "var_s")
        std_s = T([128, 1], "std_s")
        rstd_s = T([128, 1], "rstd_s")
        scale_s = T([128, 1], "scale_s")
        negscale = T([128, 1], "negscale")
        beff = T([128, 1], "beff")
        bkk = T([128, 1], "bkk")
        bqq = T([128, 1], "bqq")
        bvr = T([128, 1], "bvr")
        eps_s = T([128, 1], "eps_s")

        sy = nc.sync

        # ---- load constants (split across the two DGE queues) ----
        nc.scalar.dma_start(cw[:, 0:4 * 128], conv_wt[:, 0:4 * 128])
        nc.scalar.dma_start(cw[:, 4 * 128:], conv_wt[:, 4 * 128:])
        sy.dma_start(gmat_s[:], gmat[:])
        sy.dma_start(wkk_s[:], wkk[:])
        sy.dma_start(wqq_s[:], wqq[:])
        sy.dma_start(wvr_s[:], wvr[:])
        sy.dma_start(proj_b_s[:], proj_b[:])
        sy.dma_start(ident_s[:], ident[:])
        sy.dma_start(conv_b_s[:], conv_b[:])
        sy.dma_start(gn_w_s[:], gn_w[:])
        sy.dma_start(gn_b_s[:], gn_b[:])
        nc.vector.memset(vaug[:], 1.0)
        nc.vector.memset(eps_s[:], EPS)
        # dummy Ln pins the natural_log_exp act-table set (holds both ln
        # and exp) during the DMA wait, avoiding mid-kernel table reloads
        nc.scalar.activation(std_s[:], eps_s[:], Act.Ln)

        import contextlib
        loop_cm = tc.For_i(0, loop_n, 1) if loop_n else contextlib.nullcontext()
        loop_cm.__enter__()

        # ---- padded image ships pre-built from host (top: reflect-padded,
        # bottom: same shifted one element for the paired conv taps).
        # Split the transfer so the first conv quad starts sooner.
        x2f = x2[:].rearrange("p a b -> p (a b)")
        SPLIT = 37 * HP
        E = HP * HP
        sy.dma_start(x2f[0:C, 0:SPLIT], x_in[:, 0:SPLIT])
        sy.dma_start(x2f[C:128, 0:SPLIT], x_in[:, 1:SPLIT + 1])
        nc.scalar.dma_start(x2f[0:C, SPLIT:E], x_in[:, SPLIT:E])
        nc.scalar.dma_start(x2f[C:128, SPLIT:E - 1], x_in[:, SPLIT + 1:E])

        # ---- depthwise convs as diagonal matmuls + fused leaky/bias/sums ----
        # block pairs: 2 PSUM banks per set, two sets in flight so the
        # evacuation of one set hides under the next set's matmuls
        with tc.tile_pool(name="cvp", bufs=2, space="PSUM") as pps, \
             tc.tile_pool(name="psb", bufs=3) as psb:
            for jq in range(NBLK // 2):
                cpss = [pps.tile([128, 2, BLK], f32, tag="conv",
                                 name=f"cps{jq}")]
                cps = cpss[0]
                for g, (dy, dx0, paired) in enumerate(TAP_GROUPS):
                    for i in range(2):
                        j = jq * 2 + i
                        lhs = cw[:, g * 128:(g + 1) * 128]
                        rows = slice(dy + 8 * j, dy + 8 * j + 8)
                        rhs = x2[:, rows, dx0:dx0 + W]
                        if not paired:
                            lhs = cw[0:C, g * 128:(g + 1) * 128]
                            rhs = x2[0:C, rows, dx0:dx0 + W]
                        nc.tensor.matmul(cps[:, i, :], lhs, rhs,
                                         start=(g == 0), stop=(g == NG - 1))
                for i in range(2):
                    j = jq * 2 + i
                    # bias add (ScalarE), leaky relu + per-channel sums (DVE)
                    ub = psb.tile([128, BLK], f32, tag="ub")
                    nc.scalar.add(ub[:], cps[:, i, :], conv_b_s[:, 0:1])
                    nc.vector.scalar_tensor_tensor(
                        out=xc[:, j * BLK:(j + 1) * BLK], in0=ub[:],
                        scalar=0.01, in1=ub[:], op0=Alu.mult, op1=Alu.max,
                        accum_out=s1p[:, j:j + 1])
            # sum of squares per channel
            for j in range(NBLK):
                dump = psb.tile([128, BLK], f32, tag="dump")
                xb = xc[:, j * BLK:(j + 1) * BLK].bitcast(f32)
                nc.vector.scalar_tensor_tensor(
                    out=dump[:], in0=xb, scalar=1.0, in1=xb,
                    op0=Alu.mult, op1=Alu.mult, accum_out=s2p[:, j:j + 1])

            nc.vector.tensor_reduce(stats[:, 0:1], s1p[:],
                                    axis=mybir.AxisListType.X, op=Alu.add)
            nc.vector.tensor_reduce(stats[:, 1:2], s2p[:],
                                    axis=mybir.AxisListType.X, op=Alu.add)

        # ---- group-norm statistics (fold into projection weights) ----
        with tc.tile_pool(name="stp", bufs=2, space="PSUM") as pps, \
             tc.tile_pool(name="psb2", bufs=2) as psb:
            gps = pps.tile([128, 2], f32, tag="gstat", bufs=1)
            nc.tensor.matmul(gps[:], gmat_s[:], stats[:, 0:2])
            # negvar = mean^2 - m2 ; ln(var+eps) = Ln(-negvar + eps)
            nc.vector.tensor_copy(mean_s[:], gps[:, 0:1])
            nc.vector.scalar_tensor_tensor(
                out=var_s[:], in0=mean_s[:], scalar=mean_s[:, 0:1],
                in1=gps[:, 1:2], op0=Alu.mult, op1=Alu.subtract)
            nc.scalar.activation(std_s[:], var_s[:], Act.Ln, scale=-1.0,
                                 bias=eps_s[:, 0:1])
            nc.scalar.activation(rstd_s[:], std_s[:], Act.Exp, scale=-0.5)
            nc.vector.tensor_mul(scale_s[:], rstd_s[:], gn_w_s[:])
            nc.vector.tensor_scalar_mul(negscale[:], scale_s[:], -1.0)
            nc.vector.scalar_tensor_tensor(
                out=beff[:], in0=mean_s[:], scalar=negscale[:, 0:1],
                in1=gn_b_s[:], op0=Alu.mult, op1=Alu.add)

            # fold GN scale into weights; GN bias via N=1 matmuls
            nc.vector.tensor_scalar_mul(wkk_e[:], wkk_s[:], scale_s[:, 0:1])
            nc.vector.tensor_scalar_mul(wqq_e[:], wqq_s[:], scale_s[:, 0:1])
            nc.vector.tensor_scalar_mul(wvr_e[:], wvr_s[:], scale_s[:, 0:1])
            for wsb, bsb in ((wkk_s, bkk), (wqq_s, bqq), (wvr_s, bvr)):
                bps = pps.tile([128, 1], f32, tag="bias")
                nc.tensor.matmul(bps[:], wsb[:], beff[:])
                nc.vector.tensor_copy(bsb[:], bps[:])

        # ---- v/residual projection, then V^T transposes (k/q production
        # for blocks >= 1 is interleaved into the first attention block's
        # chunk loop; block 0 is produced here so scoring starts early) ----
        with tc.tile_pool(name="vrp", bufs=4, space="PSUM") as pps, \
             tc.tile_pool(name="trp", bufs=4, space="PSUM") as tpp:
            kps = pps.tile([128, BLK], f32, tag="kq", name="kps0")
            nc.tensor.matmul(kps[:], wkk_e[:], xc[:, 0:BLK])
            nc.vector.tensor_scalar_add(kk[:, 0:BLK], kps[:], bkk[:, 0:1])
            qps = pps.tile([128, BLK], f32, tag="kq", name="qps0")
            nc.tensor.matmul(qps[:], wqq_e[:], xc[:, 0:BLK])
            nc.vector.tensor_scalar_add(qq[:, 0:BLK], qps[:], bqq[:, 0:1])
            for j in range(NBLK):
                blk = slice(j * BLK, (j + 1) * BLK)
                vps = pps.tile([128, BLK], f32, tag="kq")
                nc.tensor.matmul(vps[:], wvr_e[:], xc[:, blk])
                nc.vector.tensor_scalar_add(vres[:, blk], vps[:],
                                            bvr[:, 0:1])
                for t in range(4 * j, 4 * j + 4):
                    tps = tpp.tile([128, C], f32, tag="tr")
                    nc.tensor.transpose(tps[:],
                                        vres[C:128, t * 128:(t + 1) * 128],
                                        ident_s[C:128, C:128])
                    nc.vector.tensor_copy(vaug[:, t * 65:t * 65 + C], tps[:])

        if dbg:
            sy.dma_start(dbg_d["d_xc"][:], xc[:].bitcast(f32))
            sy.dma_start(dbg_d["d_kk"][:], kk[:].bitcast(f32))
            sy.dma_start(dbg_d["d_qq"][:], qq[:].bitcast(f32))
            sy.dma_start(dbg_d["d_vres"][:], vres[:])
            sy.dma_start(dbg_d["d_vaug"][:], vaug[:].bitcast(f32))
            sy.dma_start(dbg_d["d_scale"][:], scale_s[:])
            sy.dma_start(dbg_d["d_beff"][:], beff[:])

        # ---- attention: one q-block at a time; k/q projection production
        # is interleaved into the first block's chunk loop so it hides
        # under the exp stream ----
        with tc.tile_pool(name="aps", bufs=2, space="PSUM") as aps, \
             tc.tile_pool(name="ops", bufs=2, space="PSUM") as ops, \
             tc.tile_pool(name="kqp", bufs=2, space="PSUM") as kqp, \
             tc.tile_pool(name="asb", bufs=2) as asb:
            i16 = mybir.dt.int16

            def emit_av(ob, cp, et, st, sp):
                c0, c1 = 2 * cp, 2 * cp + 1
                nc.tensor.matmul(ob[:],
                                 vaug[:, c0 * 65:c0 * 65 + 65],
                                 et[:, 0:BLK], start=st, stop=False)
                nc.tensor.matmul(ob[:],
                                 vaug[:, c1 * 65:c1 * 65 + 65],
                                 et[:, BLK:2 * BLK],
                                 start=False, stop=sp)

            for j in range(NBLK):
                ob = ops.tile([65, BLK], f32, tag="oacc", name=f"o{j}")
                qs = slice(j * BLK, (j + 1) * BLK)
                pend = []  # (cp, et) pairs awaiting their AV matmuls
                for cp in range(NCHUNK // 2):
                    c0, c1 = 2 * cp, 2 * cp + 1
                    k0 = slice(c0 * CHUNK, (c0 + 1) * CHUNK)
                    k1 = slice(c1 * CHUNK, (c1 + 1) * CHUNK)
                    if j == 0 and cp % 2 == 1 and cp < 15:
                        b = (cp + 1) // 2
                        bb = slice(b * BLK, (b + 1) * BLK)
                        kps = kqp.tile([128, BLK], f32, tag="kq")
                        nc.tensor.matmul(kps[:], wkk_e[:], xc[:, bb])
                        nc.vector.tensor_scalar_add(kk[:, bb], kps[:],
                                                    bkk[:, 0:1])
                        qps = kqp.tile([128, BLK], f32, tag="kq")
                        nc.tensor.matmul(qps[:], wqq_e[:], xc[:, bb])
                        nc.vector.tensor_scalar_add(qq[:, bb], qps[:],
                                                    bqq[:, 0:1])
                    stq = aps.tile([128, 2 * BLK], f32, tag="stq")
                    # even chunk on PE rows 0-63, odd on rows 64-127
                    nc.tensor.matmul(stq[:, 0:BLK],
                                     kk[0:C, k0], qq[0:C, qs])
                    nc.tensor.matmul(stq[:, BLK:2 * BLK],
                                     kk[C:128, k1], qq[C:128, qs])
                    et = asb.tile([128, 2 * BLK], bf16, tag="expst",
                                  bufs=6)
                    if cp not in DVE_CPS:
                        nc.scalar.activation(et[:], stq[:], Act.Exp,
                                             scale=0.125)
                    else:
                        nc.vector.tensor_scalar(
                            out=et[:].bitcast(i16), in0=stq[:],
                            scalar1=A16 * 0.125, scalar2=B16,
                            op0=Alu.mult, op1=Alu.add)
                    # software pipeline, lag 2: AV for pair cp-2 issues
                    # after this pair's scores, so the in-order PE stream
                    # never stalls waiting for an exp to finish
                    pend.append((cp, et))
                    if len(pend) > 2:
                        pcp, pet = pend.pop(0)
                        emit_av(ob, pcp, pet, pcp == 0, False)
                for pcp, pet in pend:
                    emit_av(ob, pcp, pet, pcp == 0,
                            pcp == NCHUNK // 2 - 1)
                # tail: normalize via reciprocal of the ones-column sums,
                # broadcast across partitions, bias+residual.
                if True:
                    blk = slice(j * BLK, (j + 1) * BLK)
                    # evacuate O quickly so its PSUM bank frees for the
                    # next block pair; finish the tail from SBUF
                    osb = asb.tile([C, BLK], f32, tag="osb")
                    nc.vector.tensor_copy(osb[:], ob[0:C, :])
                    ss = asb.tile([1, BLK], f32, tag="ss")
                    nc.vector.tensor_copy(ss[:], ob[C:C + 1, :])
                    rs = asb.tile([1, BLK], f32, tag="rs")
                    nc.vector.reciprocal_approx_fast(rs[:], ss[:])
                    rbs = asb.tile([C, BLK], f32, tag="rbs")
                    nc.gpsimd.partition_broadcast(rbs[:], rs[:])
                    if dbg and j == 0:
                        sy.dma_start(dbg_d["d_rs"][:], rs[:])
                        sy.dma_start(dbg_d["d_rbs"][:], rbs[:])
                    tmp = asb.tile([C, BLK], f32, tag="tmp")
                    nc.vector.tensor_mul(tmp[:], osb[:], rbs[:])
                    ot = asb.tile([C, BLK], f32, tag="ot")
                    nc.vector.scalar_tensor_tensor(
                        out=ot[:], in0=tmp[:], scalar=proj_b_s[:, 0:1],
                        in1=vres[0:C, blk], op0=Alu.add, op1=Alu.add)
                    sy.dma_start(out_d[:, blk], ot[:])

        loop_cm.__exit__(None, None, None)
        persist.release()

    nc.compile()
    return nc


def _host_prep(inputs):
    """Build per-core input maps from the full inputs."""
    x = np.ascontiguousarray(inputs["inputs"], np.float32)  # [B, C, H, W]
    dw1 = np.asarray(inputs["dw1_w"], np.float32).reshape(C, 3, 3)
    dw2 = np.asarray(inputs["dw2_w"], np.float32).reshape(C, 5, 5)
    w3e = np.zeros((C, 5, 5), np.float32)
    w3e[:, 1:4, 1:4] = dw1
    conv_wt = np.zeros((128, NG, 128), np.float32)
    cidx = np.arange(C)
    for g, (dy, dx0, paired) in enumerate(TAP_GROUPS):
        conv_wt[cidx, g, cidx] = w3e[:, dy, dx0]
        conv_wt[cidx, g, C + cidx] = dw2[:, dy, dx0]
        if paired:
            conv_wt[C + cidx, g, cidx] = w3e[:, dy, dx0 + 1]
            conv_wt[C + cidx, g, C + cidx] = dw2[:, dy, dx0 + 1]
    conv_wt = conv_wt.reshape(128, NG * 128)

    conv_b = np.concatenate([inputs["dw1_b"], inputs["dw2_b"]]).astype(
        np.float32).reshape(128, 1)
    gn_w = np.concatenate([inputs["gnA_w"], inputs["gnB_w"]]).astype(
        np.float32).reshape(128, 1)
    gn_b = np.concatenate([inputs["gnA_b"], inputs["gnB_b"]]).astype(
        np.float32).reshape(128, 1)

    gmat = np.zeros((128, 128), np.float32)
    cpg = C // GROUPS  # 4
    npix = cpg * HW
    for k in range(128):
        g0 = k // cpg
        gmat[k, g0 * cpg:(g0 + 1) * cpg] = 1.0 / npix
    gmat = gmat.T.copy()  # lhsT[k, m]: symmetric anyway, but be explicit

    qkvA = np.asarray(inputs["qkvA_w"], np.float32)
    qkvB = np.asarray(inputs["qkvB_w"], np.float32)
    ident = np.eye(128, dtype=np.float32)

    in_maps = []
    xpads = {}
    for b in range(B):
        xpads[b] = np.ascontiguousarray(
            np.pad(x[b], ((0, 0), (PAD, PAD), (PAD, PAD)),
                   mode="reflect").reshape(C, HP * HP))
    for core in range(N_CORES):
        b, br = core // 2, core % 2
        wkk = np.zeros((128, 128), np.float32)
        wqq = np.zeros((128, 128), np.float32)
        wvr = np.zeros((128, 128), np.float32)
        if br == 0:  # out_A: k,v from branch1 (x_A), q from branch2 (x_B)
            wkk[0:C, 0:C] = qkvA[C:2 * C, :].T
            wkk[0:C, C:128] = qkvA[C:2 * C, :].T
            wqq[C:128, 0:C] = qkvB[0:C, :].T
            wqq[C:128, C:128] = qkvB[0:C, :].T
            wvr[0:C, 0:C] = np.eye(C)  # residual = x_A
            # fold the output projection into V: AV matmul then yields
            # the already-projected attention output
            pw, pb = inputs["outA_w"], inputs["outA_b"]
            wvr[0:C, C:128] = (np.asarray(pw, np.float64) @
                               np.asarray(qkvA[2 * C:3 * C, :], np.float64)).T
        else:  # out_B: k,v from branch2 (x_B), q from branch1 (x_A)
            wkk[C:128, 0:C] = qkvB[C:2 * C, :].T
            wkk[C:128, C:128] = qkvB[C:2 * C, :].T
            wqq[0:C, 0:C] = qkvA[0:C, :].T
            wqq[0:C, C:128] = qkvA[0:C, :].T
            wvr[C:128, 0:C] = np.eye(C)  # residual = x_B
            pw, pb = inputs["outB_w"], inputs["outB_b"]
            wvr[C:128, C:128] = (np.asarray(pw, np.float64) @
                                 np.asarray(qkvB[2 * C:3 * C, :], np.float64)).T
        in_maps.append({
            "x_in": xpads[b],
            "conv_wt": conv_wt,
            "conv_b": conv_b,
            "gn_w": gn_w,
            "gn_b": gn_b,
            "gmat": gmat,
            "wkk": wkk,
            "wqq": wqq,
            "wvr": wvr,
            "proj_b": np.asarray(pb, np.float32).reshape(C, 1),
            "ident": ident,
        })
    return in_maps


def kernel(**inputs) -> np.ndarray:
    global _compiled, last_res
    import os
    from concourse.bass_utils import run_bass_kernel_spmd

    if _compiled is None:
        _compiled = _build()
    in_maps = _host_prep(inputs)
    res = run_bass_kernel_spmd(_compiled, in_maps, core_ids=list(range(N_CORES)),
                               tmpdir=os.environ.get("KTRACE_DIR") or None)
    last_res = res
    out = np.empty((B, 2 * C, H, W), np.float32)
    for core in range(N_CORES):
        b, br = core // 2, core % 2
        out[b, br * C:(br + 1) * C] = res.results[core]["out"].reshape(C, H, W)
    return out


# revision 38
# speedup vs baseline: 1.0019x; 1.0019x over previous
"""Trainium2 Bass kernel for nn_Cross_attention_89730456748795.

Full-input contract: kernel(**inputs) takes the unsharded inputs of
reference.setup_inputs() and returns the full [4, 128, 64, 64] output.

Sharding: the model has 4 batches x 2 cross-attention branches = 8
independent attention instances; one per NeuronCore (pure data parallel,
no collectives). Each core:
  - runs both depthwise convs (3x3 reflect-pad-1 and 5x5 reflect-pad-2)
    as diagonal-weight PE matmuls over a reflect-padded image, with
    LeakyReLU+bias+per-channel sums fused into one ScalarE op per tile,
  - folds both GroupNorms into the QKV projection weights (scale into the
    stationary operand, bias via a tiny N=1 matmul),
  - computes one full 4096x4096 attention in scores-transposed layout
    (softmax denominators come free from a ones-column appended to V^T),
  - output-projects, divides by the softmax sums, adds bias + residual.
"""

import math

import numpy as np

B, C, H, W = 4, 64, 64, 64
HW = H * W  # 4096
PAD = 2
HP = H + 2 * PAD  # 68
N_CORES = 8
GROUPS = 16
EPS = 1e-5
NBLK = 8  # q blocks of 512
BLK = 512
NCHUNK = 32  # k chunks of 128
CHUNK = 128
# chunk-pairs whose exp runs on DVE via the Schraudolph bit trick
# (i16 = round(A16*arg + B16) bitcast to bf16 ~= exp(arg), 1.8% rms),
# offloading ~1/4 of the exp stream from the bottleneck ScalarE
DVE_CPS = (3, 7, 11, 15)
A16 = 128.0 / math.log(2.0)
B16 = 16248.5

# conv tap groups: (dy, dx0, paired). Pairs cover (dy,dx0),(dy,dx0+1).
TAP_GROUPS = [(dy, dx0, True) for dy in range(5) for dx0 in (0, 2)] + [
    (dy, 4, False) for dy in range(5)
]
NG = len(TAP_GROUPS)  # 15

_compiled = None
last_res = None  # BassKernelResults of the most recent run (for profiling)


def _build(dbg=False, loop_n=None):
    import concourse.tile as tile
    from concourse import bacc, mybir

    f32 = mybir.dt.float32
    f32r = mybir.dt.float32r
    Alu = mybir.AluOpType
    Act = mybir.ActivationFunctionType

    nc = bacc.Bacc("TRN2", target_bir_lowering=False, debug=False,
                   num_devices=N_CORES)

    def din(name, shape, dt=f32):
        return nc.dram_tensor(name, shape, dt, kind="ExternalInput").ap()

    x_in = din("x_in", [C, HP * HP], f32r)
    conv_wt = din("conv_wt", [128, NG * 128], f32r)
    conv_b = din("conv_b", [128, 1])
    gn_w = din("gn_w", [128, 1])
    gn_b = din("gn_b", [128, 1])
    gmat = din("gmat", [128, 128])
    wkk = din("wkk", [128, 128])
    wqq = din("wqq", [128, 128])
    wvr = din("wvr", [128, 128])
    proj_b = din("proj_b", [C, 1])
    ident = din("ident", [128, 128])
    out_d = nc.dram_tensor("out", [C, HW], f32, kind="ExternalOutput").ap()
    dbg_d = {}
    if dbg:
        for nm, shp in [("d_xc", [128, HW]), ("d_kk", [128, HW]),
                        ("d_qq", [128, HW]), ("d_vres", [128, HW]),
                        ("d_vaug", [128, NCHUNK * 65]),
                        ("d_scale", [128, 1]), ("d_beff", [128, 1]),
                        ("d_rs", [1, BLK]),
                        ("d_rbs", [C, BLK]), ("d_et", [128, 2 * BLK])]:
            dbg_d[nm] = nc.dram_tensor(nm, shp, f32,
                                       kind="ExternalOutput").ap()

    with tile.TileContext(nc) as tc:
        # ---- persistent SBUF tensors ----
        persist = tc.alloc_tile_pool(name="persist", bufs=1)

        def T(shape, name, dt=f32):
            return persist.tile(shape, dt, tag=name, name=name)

        x2 = T([128, HP, HP], "x2", f32r)  # top: padded, bottom: +1 shift
        cw = T([128, NG * 128], "cw", f32r)
        xc = T([128, HW], "xc", f32r)  # conv+leaky out (br1|br2)
        bf16 = mybir.dt.bfloat16
        kk = T([128, HW], "kk", bf16)  # kf duplicated on both halves
        qq = T([128, HW], "qq", bf16)  # qf duplicated on both halves
        vres = T([128, HW], "vres")  # rows 0-63 res, 64-127 vf
        vaug = T([128, NCHUNK * 65], "vaug", bf16)
        gmat_s = T([128, 128], "gmat_s")
        wkk_s = T([128, 128], "wkk_s")
        wqq_s = T([128, 128], "wqq_s")
        wvr_s = T([128, 128], "wvr_s")
        wkk_e = T([128, 128], "wkk_e", f32r)
        wqq_e = T([128, 128], "wqq_e", f32r)
        wvr_e = T([128, 128], "wvr_e", f32r)
        proj_b_s = T([C, 1], "proj_b_s")
        ident_s = T([128, 128], "ident_s")
        conv_b_s = T([128, 1], "conv_b_s")
        gn_w_s = T([128, 1], "gn_w_s")
        gn_b_s = T([128, 1], "gn_b_s")
        s1p = T([128, NBLK], "s1p")
        s2p = T([128, NBLK], "s2p")
        stats = T([128, 2], "stats")
        mean_s = T([128, 1], "mean_s")
        negmean = T([128, 1], "negmean")
        var_s = T([128, 1], "var_s")
        std_s = T([128, 1], "std_s")
        rstd_s = T([128, 1], "rstd_s")
        scale_s = T([128, 1], "scale_s")
        negscale = T([128, 1], "negscale")
        beff = T([128, 1], "beff")
        bkk = T([128, 1], "bkk")
        bqq = T([128, 1], "bqq")
        bvr = T([128, 1], "bvr")
        eps_s = T([128, 1], "eps_s")

        sy = nc.sync

        # ---- load constants (split across the two DGE queues) ----
        nc.scalar.dma_start(cw[:, 0:4 * 128], conv_wt[:, 0:4 * 128])
        nc.scalar.dma_start(cw[:, 4 * 128:], conv_wt[:, 4 * 128:])
        sy.dma_start(gmat_s[:], gmat[:])
        sy.dma_start(wkk_s[:], wkk[:])
        sy.dma_start(wqq_s[:], wqq[:])
        sy.dma_start(wvr_s[:], wvr[:])
        sy.dma_start(proj_b_s[:], proj_b[:])
        sy.dma_start(ident_s[:], ident[:])
        sy.dma_start(conv_b_s[:], conv_b[:])
        sy.dma_start(gn_w_s[:], gn_w[:])
        sy.dma_start(gn_b_s[:], gn_b[:])
        nc.vector.memset(vaug[:], 1.0)
        nc.vector.memset(eps_s[:], EPS)
        # dummy Ln pins the natural_log_exp act-table set (holds both ln
        # and exp) during the DMA wait, avoiding mid-kernel table reloads
        nc.scalar.activation(std_s[:], eps_s[:], Act.Ln)

        import contextlib
        loop_cm = tc.For_i(0, loop_n, 1) if loop_n else contextlib.nullcontext()
        loop_cm.__enter__()

        # ---- padded image ships pre-built from host (top: reflect-padded,
        # bottom: same shifted one element for the paired conv taps).
        # Split the transfer so the first conv quad starts sooner.
        x2f = x2[:].rearrange("p a b -> p (a b)")
        SPLIT = 37 * HP
        E = HP * HP
        sy.dma_start(x2f[0:C, 0:SPLIT], x_in[:, 0:SPLIT])
        sy.dma_start(x2f[C:128, 0:SPLIT], x_in[:, 1:SPLIT + 1])
        nc.scalar.dma_start(x2f[0:C, SPLIT:E], x_in[:, SPLIT:E])
        nc.scalar.dma_start(x2f[C:128, SPLIT:E - 1], x_in[:, SPLIT + 1:E])

        # ---- depthwise convs as diagonal matmuls + fused leaky/bias/sums ----
        # block pairs: 2 PSUM banks per set, two sets in flight so the
        # evacuation of one set hides under the next set's matmuls
        with tc.tile_pool(name="cvp", bufs=2, space="PSUM") as pps, \
             tc.tile_pool(name="psb", bufs=3) as psb:
            for jq in range(NBLK // 2):
                cpss = [pps.tile([128, 2, BLK], f32, tag="conv",
                                 name=f"cps{jq}")]
                cps = cpss[0]
                for g, (dy, dx0, paired) in enumerate(TAP_GROUPS):
                    for i in range(2):
                        j = jq * 2 + i
                        lhs = cw[:, g * 128:(g + 1) * 128]
                        rows = slice(dy + 8 * j, dy + 8 * j + 8)
                        rhs = x2[:, rows, dx0:dx0 + W]
                        if not paired:
                            lhs = cw[0:C, g * 128:(g + 1) * 128]
                            rhs = x2[0:C, rows, dx0:dx0 + W]
                        nc.tensor.matmul(cps[:, i, :], lhs, rhs,
                                         start=(g == 0), stop=(g == NG - 1))
                for i in range(2):
                    j = jq * 2 + i
                    # bias add (ScalarE), leaky relu + per-channel sums (DVE)
                    ub = psb.tile([128, BLK], f32, tag="ub")
                    nc.scalar.add(ub[:], cps[:, i, :], conv_b_s[:, 0:1])
                    nc.vector.scalar_tensor_tensor(
                        out=xc[:, j * BLK:(j + 1) * BLK], in0=ub[:],
                        scalar=0.01, in1=ub[:], op0=Alu.mult, op1=Alu.max,
                        accum_out=s1p[:, j:j + 1])
            # sum of squares per channel
            for j in range(NBLK):
                dump = psb.tile([128, BLK], f32, tag="dump")
                xb = xc[:, j * BLK:(j + 1) * BLK].bitcast(f32)
                nc.vector.scalar_tensor_tensor(
                    out=dump[:], in0=xb, scalar=1.0, in1=xb,
                    op0=Alu.mult, op1=Alu.mult, accum_out=s2p[:, j:j + 1])

            nc.vector.tensor_reduce(stats[:, 0:1], s1p[:],
                                    axis=mybir.AxisListType.X, op=Alu.add)
            nc.vector.tensor_reduce(stats[:, 1:2], s2p[:],
                                    axis=mybir.AxisListType.X, op=Alu.add)

        # ---- group-norm statistics (fold into projection weights) ----
        with tc.tile_pool(name="stp", bufs=2, space="PSUM") as pps, \
             tc.tile_pool(name="psb2", bufs=2) as psb:
            gps = pps.tile([128, 2], f32, tag="gstat", bufs=1)
            nc.tensor.matmul(gps[:], gmat_s[:], stats[:, 0:2])
            # negvar = mean^2 - m2 ; ln(var+eps) = Ln(-negvar + eps)
            nc.vector.tensor_copy(mean_s[:], gps[:, 0:1])
            nc.vector.scalar_tensor_tensor(
                out=var_s[:], in0=mean_s[:], scalar=mean_s[:, 0:1],
                in1=gps[:, 1:2], op0=Alu.mult, op1=Alu.subtract)
            nc.scalar.activation(std_s[:], var_s[:], Act.Ln, scale=-1.0,
                                 bias=eps_s[:, 0:1])
            nc.scalar.activation(rstd_s[:], std_s[:], Act.Exp, scale=-0.5)
            nc.vector.tensor_mul(scale_s[:], rstd_s[:], gn_w_s[:])
            nc.vector.tensor_scalar_mul(negscale[:], scale_s[:], -1.0)
            nc.vector.scalar_tensor_tensor(
                out=beff[:], in0=mean_s[:], scalar=negscale[:, 0:1],
                in1=gn_b_s[:], op0=Alu.mult, op1=Alu.add)

            # fold GN scale into weights; GN bias via N=1 matmuls
            nc.vector.tensor_scalar_mul(wkk_e[:], wkk_s[:], scale_s[:, 0:1])
            nc.vector.tensor_scalar_mul(wqq_e[:], wqq_s[:], scale_s[:, 0:1])
            nc.vector.tensor_scalar_mul(wvr_e[:], wvr_s[:], scale_s[:, 0:1])
            for wsb, bsb in ((wkk_s, bkk), (wqq_s, bqq), (wvr_s, bvr)):
                bps = pps.tile([128, 1], f32, tag="bias")
                nc.tensor.matmul(bps[:], wsb[:], beff[:])
                nc.vector.tensor_copy(bsb[:], bps[:])

        # ---- v/residual projection, then V^T transposes (k/q production
        # for blocks >= 1 is interleaved into the first attention block's
        # chunk loop; block 0 is produced here so scoring starts early) ----
        with tc.tile_pool(name="vrp", bufs=4, space="PSUM") as pps, \
             tc.tile_pool(name="trp", bufs=4, space="PSUM") as tpp:
            kps = pps.tile([128, BLK], f32, tag="kq", name="kps0")
            nc.tensor.matmul(kps[:], wkk_e[:], xc[:, 0:BLK])
            nc.vector.tensor_scalar_add(kk[:, 0:BLK], kps[:], bkk[:, 0:1])
            qps = pps.tile([128, BLK], f32, tag="kq", name="qps0")
            nc.tensor.matmul(qps[:], wqq_e[:], xc[:, 0:BLK])
            nc.vector.tensor_scalar_add(qq[:, 0:BLK], qps[:], bqq[:, 0:1])
            for j in range(NBLK):
                blk = slice(j * BLK, (j + 1) * BLK)
                vps = pps.tile([128, BLK], f32, tag="kq")
                nc.tensor.matmul(vps[:], wvr_e[:], xc[:, blk])
                nc.vector.tensor_scalar_add(vres[:, blk], vps[:],
                                            bvr[:, 0:1])
                for t in range(4 * j, 4 * j + 4):
                    tps = tpp.tile([128, C], f32, tag="tr")
                    nc.tensor.transpose(tps[:],
                                        vres[C:128, t * 128:(t + 1) * 128],
                                        ident_s[C:128, C:128])
                    nc.vector.tensor_copy(vaug[:, t * 65:t * 65 + C], tps[:])

        if dbg:
            sy.dma_start(dbg_d["d_xc"][:], xc[:].bitcast(f32))
            sy.dma_start(dbg_d["d_kk"][:], kk[:].bitcast(f32))
            sy.dma_start(dbg_d["d_qq"][:], qq[:].bitcast(f32))
            sy.dma_start(dbg_d["d_vres"][:], vres[:])
            sy.dma_start(dbg_d["d_vaug"][:], vaug[:].bitcast(f32))
            sy.dma_start(dbg_d["d_scale"][:], scale_s[:])
            sy.dma_start(dbg_d["d_beff"][:], beff[:])

        # ---- attention: one q-block at a time; k/q projection production
        # is interleaved into the first block's chunk loop so it hides
        # under the exp stream ----
        with tc.tile_pool(name="aps", bufs=2, space="PSUM") as aps, \
             tc.tile_pool(name="ops", bufs=2, space="PSUM") as ops, \
             tc.tile_pool(name="kqp", bufs=2, space="PSUM") as kqp, \
             tc.tile_pool(name="asb", bufs=2) as asb:
            i16 = mybir.dt.int16

            def emit_av(ob, cp, et, st, sp):
                c0, c1 = 2 * cp, 2 * cp + 1
                nc.tensor.matmul(ob[:],
                                 vaug[:, c0 * 65:c0 * 65 + 65],
                                 et[:, 0:BLK], start=st, stop=False)
                nc.tensor.matmul(ob[:],
                                 vaug[:, c1 * 65:c1 * 65 + 65],
                                 et[:, BLK:2 * BLK],
                                 start=False, stop=sp)

            for j in range(NBLK):
                ob = ops.tile([65, BLK], f32, tag="oacc", name=f"o{j}")
                qs = slice(j * BLK, (j + 1) * BLK)
                pend = []  # (cp, et) pairs awaiting their AV matmuls
                for cp in range(NCHUNK // 2):
                    c0, c1 = 2 * cp, 2 * cp + 1
                    k0 = slice(c0 * CHUNK, (c0 + 1) * CHUNK)
                    k1 = slice(c1 * CHUNK, (c1 + 1) * CHUNK)
                    if j == 0 and cp % 2 == 1 and cp < 15:
                        b = (cp + 1) // 2
                        bb = slice(b * BLK, (b + 1) * BLK)
                        kps = kqp.tile([128, BLK], f32, tag="kq")
                        nc.tensor.matmul(kps[:], wkk_e[:], xc[:, bb])
                        nc.vector.tensor_scalar_add(kk[:, bb], kps[:],
                                                    bkk[:, 0:1])
                        qps = kqp.tile([128, BLK], f32, tag="kq")
                        nc.tensor.matmul(qps[:], wqq_e[:], xc[:, bb])
                        nc.vector.tensor_scalar_add(qq[:, bb], qps[:],
                                                    bqq[:, 0:1])
                    stq = aps.tile([128, 2 * BLK], f32, tag="stq")
                    # even chunk on PE rows 0-63, odd on rows 64-127
                    nc.tensor.matmul(stq[:, 0:BLK],
                                     kk[0:C, k0], qq[0:C, qs])
                    nc.tensor.matmul(stq[:, BLK:2 * BLK],
                                     kk[C:128, k1], qq[C:128, qs])
                    et = asb.tile([128, 2 * BLK], bf16, tag="expst",
                                  bufs=6)
                    if cp not in DVE_CPS:
                        nc.scalar.activation(et[:], stq[:], Act.Exp,
                                             scale=0.125)
                    else:
                        nc.vector.tensor_scalar(
                            out=et[:].bitcast(i16), in0=stq[:],
                            scalar1=A16 * 0.125, scalar2=B16,
                            op0=Alu.mult, op1=Alu.add)
                    # software pipeline, lag 2: AV for pair cp-2 issues
                    # after this pair's scores, so the in-order PE stream
                    # never stalls waiting for an exp to finish
                    pend.append((cp, et))
                    if len(pend) > 2:
                        pcp, pet = pend.pop(0)
                        emit_av(ob, pcp, pet, pcp == 0, False)
                for pcp, pet in pend:
                    emit_av(ob, pcp, pet, pcp == 0,
                            pcp == NCHUNK // 2 - 1)
                # tail: normalize via reciprocal of the ones-column sums,
                # broadcast across partitions, bias+residual.
                if True:
                    blk = slice(j * BLK, (j + 1) * BLK)
                    # evacuate O quickly so its PSUM bank frees for the
                    # next block pair; finish the tail from SBUF
                    osb = asb.tile([C, BLK], f32, tag="osb")
                    nc.vector.tensor_copy(osb[:], ob[0:C, :])
                    ss = asb.tile([1, BLK], f32, tag="ss")
                    nc.vector.tensor_copy(ss[:], ob[C:C + 1, :])
                    rs = asb.tile([1, BLK], f32, tag="rs")
                    nc.vector.reciprocal_approx_fast(rs[:], ss[:])
                    rbs = asb.tile([C, BLK], f32, tag="rbs")
                    nc.gpsimd.partition_broadcast(rbs[:], rs[:])
                    if dbg and j == 0:
                        sy.dma_start(dbg_d["d_rs"][:], rs[:])
                        sy.dma_start(dbg_d["d_rbs"][:], rbs[:])
                    tmp = asb.tile([C, BLK], f32, tag="tmp")
                    nc.vector.tensor_mul(tmp[:], osb[:], rbs[:])
                    ot = asb.tile([C, BLK], f32, tag="ot")
                    nc.vector.scalar_tensor_tensor(
                        out=ot[:], in0=tmp[:], scalar=proj_b_s[:, 0:1],
                        in1=vres[0:C, blk], op0=Alu.add, op1=Alu.add)
                    sy.dma_start(out_d[:, blk], ot[:])

        loop_cm.__exit__(None, None, None)
        persist.release()

    nc.compile()
    return nc


def _host_prep(inputs):
    """Build per-core input maps from the full inputs."""
    x = np.ascontiguousarray(inputs["inputs"], np.float32)  # [B, C, H, W]
    dw1 = np.asarray(inputs["dw1_w"], np.float32).reshape(C, 3, 3)
    dw2 = np.asarray(inputs["dw2_w"], np.float32).reshape(C, 5, 5)
    w3e = np.zeros((C, 5, 5), np.float32)
    w3e[:, 1:4, 1:4] = dw1
    conv_wt = np.zeros((128, NG, 128), np.float32)
    cidx = np.arange(C)
    for g, (dy, dx0, paired) in enumerate(TAP_GROUPS):
        conv_wt[cidx, g, cidx] = w3e[:, dy, dx0]
        conv_wt[cidx, g, C + cidx] = dw2[:, dy, dx0]
        if paired:
            conv_wt[C + cidx, g, cidx] = w3e[:, dy, dx0 + 1]
            conv_wt[C + cidx, g, C + cidx] = dw2[:, dy, dx0 + 1]
    conv_wt = conv_wt.reshape(128, NG * 128)

    conv_b = np.concatenate([inputs["dw1_b"], inputs["dw2_b"]]).astype(
        np.float32).reshape(128, 1)
    gn_w = np.concatenate([inputs["gnA_w"], inputs["gnB_w"]]).astype(
        np.float32).reshape(128, 1)
    gn_b = np.concatenate([inputs["gnA_b"], inputs["gnB_b"]]).astype(
        np.float32).reshape(128, 1)

    gmat = np.zeros((128, 128), np.float32)
    cpg = C // GROUPS  # 4
    npix = cpg * HW
    for k in range(128):
        g0 = k // cpg
        gmat[k, g0 * cpg:(g0 + 1) * cpg] = 1.0 / npix
    gmat = gmat.T.copy()  # lhsT[k, m]: symmetric anyway, but be explicit

    qkvA = np.asarray(inputs["qkvA_w"], np.float32)
    qkvB = np.asarray(inputs["qkvB_w"], np.float32)
    ident = np.eye(128, dtype=np.float32)

    in_maps = []
    xpads = {}
    for b in range(B):
        xpads[b] = np.ascontiguousarray(
            np.pad(x[b], ((0, 0), (PAD, PAD), (PAD, PAD)),
                   mode="reflect").reshape(C, HP * HP))
    for core in range(N_CORES):
        b, br = core // 2, core % 2
        wkk = np.zeros((128, 128), np.float32)
        wqq = np.zeros((128, 128), np.float32)
        wvr = np.zeros((128, 128), np.float32)
        if br == 0:  # out_A: k,v from branch1 (x_A), q from branch2 (x_B)
            wkk[0:C, 0:C] = qkvA[C:2 * C, :].T
            wkk[0:C, C:128] = qkvA[C:2 * C, :].T
            wqq[C:128, 0:C] = qkvB[0:C, :].T
            wqq[C:128, C:128] = qkvB[0:C, :].T
            wvr[0:C, 0:C] = np.eye(C)  # residual = x_A
            # fold the output projection into V: AV matmul then yields
            # the already-projected attention output
            pw, pb = inputs["outA_w"], inputs["outA_b"]
            wvr[0:C, C:128] = (np.asarray(pw, np.float64) @
                               np.asarray(qkvA[2 * C:3 * C, :], np.float64)).T
        else:  # out_B: k,v from branch2 (x_B), q from branch1 (x_A)
            wkk[C:128, 0:C] = qkvB[C:2 * C, :].T
            wkk[C:128, C:128] = qkvB[C:2 * C, :].T
            wqq[0:C, 0:C] = qkvA[0:C, :].T
            wqq[0:C, C:128] = qkvA[0:C, :].T
            wvr[C:128, 0:C] = np.eye(C)  # residual = x_B
            pw, pb = inputs["outB_w"], inputs["outB_b"]
            wvr[C:128, C:128] = (np.asarray(pw, np.float64) @
                                 np.asarray(qkvB[2 * C:3 * C, :], np.float64)).T
        in_maps.append({
            "x_in": xpads[b],
            "conv_wt": conv_wt,
            "conv_b": conv_b,
            "gn_w": gn_w,
            "gn_b": gn_b,
            "gmat": gmat,
            "wkk": wkk,
            "wqq": wqq,
            "wvr": wvr,
            "proj_b": np.asarray(pb, np.float32).reshape(C, 1),
            "ident": ident,
        })
    return in_maps


def kernel(**inputs) -> np.ndarray:
    global _compiled, last_res
    import os
    from concourse.bass_utils import run_bass_kernel_spmd

    if _compiled is None:
        _compiled = _build()
    in_maps = _host_prep(inputs)
    res = run_bass_kernel_spmd(_compiled, in_maps, core_ids=list(range(N_CORES)),
                               tmpdir=os.environ.get("KTRACE_DIR") or None)
    last_res = res
    out = np.empty((B, 2 * C, H, W), np.float32)
    for core in range(N_CORES):
        b, br = core // 2, core % 2
        out[b, br * C:(br + 1) * C] = res.results[core]["out"].reshape(C, H, W)
    return out
